# revision 1
# baseline (speedup 1.0000x reference)
"""Trainium2 Bass kernel for nn_Attention_78554951844258.

Dense 12-head attention block: qkv = x@Wqkv+b; RoPE(q,k); softmax(q k^T/sqrt(d)) v; proj.

Sharding: data-parallel over batch — each of the 8 NeuronCores computes one
batch element end-to-end (no collectives).

Algebraic restructuring (host-side, exact, O(weights)):
  * The reference applies RoPE with seq_dim=1 on [b,h,n,d], so cos/sin depend
    only on (head, dim) — RoPE is a position-independent per-head 64x64 linear
    map M_h that folds into the q/k columns of w_qkv (and biases).
  * The softmax scale 1/sqrt(d) folds into the q weights.
  * The v bias and proj bias fold into a single output bias
    b_out = b_v @ w_proj + b_proj, because softmax rows sum to 1.
  * Softmax max-subtraction is skipped: folded scores are bounded (|S| < ~3),
    exp is safe in fp32 and the result is mathematically identical.

Device layout per core (batch element b):
  qk^T [1536, 1024] = (w_qk)^T-stationary matmuls against x^T  (c on partitions)
  v    [1024, 768]  with a ones column appended per head ("v_aug", [j, 12*65])
  S^T  [j, i] per head = k^T-stationary x q^T-moving (K=64, two heads row-packed)
  P^T  = exp(S^T) via ACT;  [out^T | colsums] = [V|1]^T-stationary x P^T-moving
  normalize out^T columns by 1/colsums (DVE recip, broadcast across
  partitions via a DRAM round-trip DMA, DVE multiply; odd head moved into its
  ovT partition range by an SBUF->SBUF DMA)
  y [i, 768] = out^T-stationary x w_proj-moving, + bias via DVE, DMA out.
Matmul operands are bf16 (weights/activations rounded on host or at the
producing engine); accumulation is fp32 in PSUM. Measured ~346 us on HW,
rel l2 err ~1.9e-3 (an fp32r variant measured 400 us at 1.2e-4).
"""
import numpy as np

NUM_HEADS = 12
E = 768
D = 64
B = 8
N = 1024
HALF = D // 2


def _ensure_axon_hooks():
    """The NTFF profile hook registry module may be missing in a fresh
    container; (re)create it so trace=True profiling degrades gracefully."""
    try:
        import antenv.axon_hooks  # noqa: F401
        return
    except ImportError:
        pass
    try:
        import antenv
        import os
        p = os.path.join(os.path.dirname(antenv.__file__), "axon_hooks.py")
        with open(p, "w") as f:
            f.write(
                "_hook = None\n\n"
                "def set_axon_ntff_profile_hook(hook):\n"
                "    global _hook\n    _hook = hook\n\n"
                "def get_axon_ntff_profile_hook():\n"
                "    return _hook\n")
    except Exception:
        pass


_ensure_axon_hooks()


# ---------------------------------------------------------------- host math
def _rope_matrix():
    """M[h, x, d]: rope(q)[x] = sum_d M[h, x, d] * q[d] (float64)."""
    inv_freq = 1.0 / (10000.0 ** (np.arange(0, D, 2, dtype=np.float64) / D))
    t = np.arange(NUM_HEADS, dtype=np.float64)
    emb = np.concatenate([t[:, None] * inv_freq[None, :]] * 2, axis=-1)  # [H, D]
    cos, sin = np.cos(emb), np.sin(emb)
    M = np.zeros((NUM_HEADS, D, D))
    for h in range(NUM_HEADS):
        for d in range(D):
            M[h, d, d] = cos[h, d]
            if d < HALF:
                M[h, d, d + HALF] = -sin[h, d]
            else:
                M[h, d, d - HALF] = sin[h, d]
    return M


def _prep_weights(w_qkv, b_qkv, w_proj, b_proj):
    w = w_qkv.astype(np.float64)
    b = b_qkv.astype(np.float64)
    M = _rope_matrix()
    scale = float(D) ** (-0.5)
    w_q = w[:, 0:E].reshape(E, NUM_HEADS, D)
    w_k = w[:, E:2 * E].reshape(E, NUM_HEADS, D)
    b_q = b[0:E].reshape(NUM_HEADS, D)
    b_k = b[E:2 * E].reshape(NUM_HEADS, D)
    w_q2 = np.einsum('ehd,hxd->ehx', w_q, M) * scale
    b_q2 = np.einsum('hd,hxd->hx', b_q, M) * scale
    w_k2 = np.einsum('ehd,hxd->ehx', w_k, M)
    b_k2 = np.einsum('hd,hxd->hx', b_k, M)
    w_qk = np.ascontiguousarray(
        np.concatenate([w_q2.reshape(E, E), w_k2.reshape(E, E)], axis=1),
        dtype=np.float32)                                     # [E, 2E]
    b_qk = np.concatenate([b_q2.reshape(E), b_k2.reshape(E)]).astype(np.float32)
    w_v = np.ascontiguousarray(w[:, 2 * E:3 * E], dtype=np.float32)
    b_out = (b[2 * E:3 * E] @ w_proj.astype(np.float64)
             + b_proj.astype(np.float64)).astype(np.float32)
    return w_qk, b_qk, w_v, b_out


# ---------------------------------------------------------------- waitfix
def _split_excess_waits(nc):
    """walrus in this container rejects >4 sync waits per instruction (and
    fewer on Drain/SP-NoOp paths). Split overflow waits onto preceding
    same-engine 1-wait NOPs — semantically identical (sequencer blocks in
    order)."""
    import concourse.mybir as mybir
    import bass_rust
    counter = [0]

    def make_nop(engine):
        counter[0] += 1
        nop = bass_rust.InstNoOp(name=f"I-waitfix-{counter[0]}", ins=[], outs=[])
        nop.engine = engine
        return nop

    for fn in nc.m.functions:
        for bb in fn.blocks:
            insts = bb.instructions
            out = []
            changed = False
            for inst in insts:
                si = inst.sync_info
                waits = list(si.on_wait) if si is not None else []
                tn = type(inst).__name__
                # Per-struct wait-slot capacity varies and several structs
                # (S3_LW matmul, S3D3_TS, SP NoOp) reject even small counts;
                # keep at most one wait everywhere, none on Drain (it gets
                # codegen-generated queue waits of its own).
                keep = 0 if tn == "InstDrain" else 1
                if len(waits) > keep:
                    for w in waits[:len(waits) - keep]:
                        nop = make_nop(inst.engine)
                        nop.sync_info = mybir.SyncInfo(on_wait=[w], on_update=[])
                        out.append(nop)
                    inst.sync_info = mybir.SyncInfo(
                        on_wait=waits[len(waits) - keep:],
                        on_update=list(si.on_update))
                    changed = True
                out.append(inst)
            if changed:
                bb.instructions = out


# ---------------------------------------------------------------- device IR
_NC_CACHE = []


def _build_nc():
    import concourse.bass as bass
    import concourse.mybir as mybir
    from concourse.tile import TileContext

    dt = mybir.dt
    f32 = dt.float32
    f32r = dt.float32r
    bf16 = dt.bfloat16
    AF = mybir.ActivationFunctionType

    nc = bass.Bass(target_bir_lowering=False)
    xT_d = nc.dram_tensor("xT", [E, N], bf16, kind="ExternalInput")
    wqk_d = nc.dram_tensor("w_qk", [E, 2 * E], bf16, kind="ExternalInput")
    bqk_d = nc.dram_tensor("b_qk", [2 * E], f32, kind="ExternalInput")
    wv_d = nc.dram_tensor("w_v", [E, E], bf16, kind="ExternalInput")
    wp_d = nc.dram_tensor("w_proj", [E, E], bf16, kind="ExternalInput")
    bo_d = nc.dram_tensor("b_out", [E], f32, kind="ExternalInput")
    y_d = nc.dram_tensor("y", [N, E], f32, kind="ExternalOutput")

    ET = E // 128          # 6 e-tiles
    IT = N // 128          # 8 i/j-tiles
    HP = NUM_HEADS // 2    # 6 head pairs

    with TileContext(nc) as tc:
        with (
            tc.tile_pool(name="phase1", bufs=1) as p1,       # xT, w_qk
            tc.tile_pool(name="persist", bufs=1) as pp,      # v_aug, w_proj, biases
            tc.tile_pool(name="psum", bufs=4, space="PSUM") as ps,
        ):
            # ---- loads
            xT = [p1.tile([128, N], bf16, tag=f"xT{e}", name=f"xT{e}")
                  for e in range(ET)]
            wqk = [p1.tile([128, 2 * E], bf16, tag=f"wqk{e}", name=f"wqk{e}")
                   for e in range(ET)]
            wp = [pp.tile([128, E], bf16, tag=f"wp{e}", name=f"wp{e}")
                  for e in range(ET)]
            for e in range(ET):
                nc.sync.dma_start(out=xT[e], in_=xT_d[e * 128:(e + 1) * 128, :])
                nc.sync.dma_start(out=wqk[e], in_=wqk_d[e * 128:(e + 1) * 128, :])
                nc.sync.dma_start(out=wp[e], in_=wp_d[e * 128:(e + 1) * 128, :])
            bq = pp.tile([128, 12], f32, tag="bq")
            nc.sync.dma_start(out=bq, in_=bqk_d[:].rearrange("(t p) -> p t", p=128))
            bo = pp.tile([128, E], f32, tag="bo")
            nc.sync.dma_start(
                out=bo,
                in_=bass.AP(tensor=bo_d[:].tensor, offset=bo_d[:].offset,
                            ap=[[0, 128], [1, E]]))

            v_aug = [pp.tile([128, NUM_HEADS * (D + 1)], bf16, tag=f"vaug{i}",
                             name=f"vaug{i}") for i in range(IT)]

            # ---- phase 1b: v = x @ w_v, written per-head with ones columns
            with tc.tile_pool(name="pwv", bufs=1) as pwv:
                wv = [pwv.tile([128, E], bf16, tag=f"wv{e}", name=f"wv{e}")
                      for e in range(ET)]
                for e in range(ET):
                    nc.sync.dma_start(out=wv[e],
                                      in_=wv_d[e * 128:(e + 1) * 128, :])
                for it in range(IT):
                    pv = ps.tile([128, E], f32, tag="ps", name=f"pv_{it}")
                    for (n0, nw) in ((0, 512), (512, 256)):
                        for e in range(ET):
                            nc.tensor.matmul(
                                pv[:, n0:n0 + nw],
                                xT[e][:, it * 128:(it + 1) * 128],
                                wv[e][:, n0:n0 + nw],
                                start=(e == 0), stop=(e == ET - 1))
                    for h in range(NUM_HEADS):
                        nc.vector.tensor_copy(
                            out=v_aug[it][:, h * 65:h * 65 + 64],
                            in_=pv[:, h * 64:(h + 1) * 64])
                    # exact 1.0 into the per-head ones columns; memset on a
                    # strided f32r view fails this walrus's ISA check, so use
                    # DVE (in0*0 + 1) with the finite bias tile as dummy input
                    ones_cols = v_aug[it].rearrange(
                        "p (h c) -> p h c", c=65)[:, :, 64:65]
                    bq12 = bq[:, 0:12].rearrange("p (a b) -> p a b", b=1)
                    nc.vector.tensor_scalar(
                        ones_cols, bq12, 0.0, 1.0,
                        mybir.AluOpType.mult, mybir.AluOpType.add)

            with (
                tc.tile_pool(name="pqk", bufs=3) as pqk,     # rotating q^T/k^T
                tc.tile_pool(name="pT", bufs=4) as ppT,      # exp'd scores
                tc.tile_pool(name="late", bufs=1) as pl,     # ovT, rc
                tc.tile_pool(name="nrm", bufs=6) as prb,      # recip broadcast
                tc.tile_pool(name="yout", bufs=2) as py,     # y staging
                tc.tile_pool(name="dscr", bufs=4, space="DRAM") as pdram,
            ):
                ovT = [pl.tile([128, N], bf16, tag=f"ovT{e}", name=f"ovT{e}")
                       for e in range(ET)]

                # ---- phase 1a interleaved with phase 2, one head pair at a time
                for hp in range(HP):
                    h0, h1 = 2 * hp, 2 * hp + 1
                    qt = pqk.tile([128, N], bf16, tag="qkT", name=f"qT{hp}")
                    kt = pqk.tile([128, N], bf16, tag="qkT", name=f"kT{hp}")
                    for ct, dst in ((hp, qt), (ET + hp, kt)):
                        pq = ps.tile([128, N], f32, tag="ps", name=f"pq_{ct}")
                        for ih in range(2):
                            for e in range(ET):
                                nc.tensor.matmul(
                                    pq[:, ih * 512:(ih + 1) * 512],
                                    wqk[e][:, ct * 128:(ct + 1) * 128],
                                    xT[e][:, ih * 512:(ih + 1) * 512],
                                    start=(e == 0), stop=(e == ET - 1))
                        nc.vector.tensor_scalar_add(dst, pq, bq[:, ct:ct + 1])

                    pv0 = ps.tile([65, N], f32, tag="ps", name=f"pv0_{hp}")
                    pv1 = ps.tile([65, N], f32, tag="ps", name=f"pv1_{hp}")
                    for jt in range(IT):
                        js = slice(jt * 128, (jt + 1) * 128)
                        st0 = ps.tile([128, N], f32, tag="ps", name=f"st0_{hp}_{jt}")
                        st1 = ps.tile([128, N], f32, tag="ps", name=f"st1_{hp}_{jt}")
                        for ih in range(2):
                            isl = slice(ih * 512, (ih + 1) * 512)
                            nc.tensor.matmul(st0[:, isl], kt[0:64, js],
                                             qt[0:64, isl])
                            nc.tensor.matmul(st1[:, isl], kt[64:128, js],
                                             qt[64:128, isl])
                        pT0 = ppT.tile([128, N], bf16, tag="pT", name=f"pT0_{hp}_{jt}")
                        pT1 = ppT.tile([128, N], bf16, tag="pT", name=f"pT1_{hp}_{jt}")
                        nc.scalar.activation(out=pT0, in_=st0, func=AF.Exp)
                        nc.scalar.activation(out=pT1, in_=st1, func=AF.Exp)
                        for ih in range(2):
                            isl = slice(ih * 512, (ih + 1) * 512)
                            nc.tensor.matmul(
                                pv0[:, isl], v_aug[jt][:, h0 * 65:h0 * 65 + 65],
                                pT0[:, isl], start=(jt == 0), stop=(jt == IT - 1))
                            nc.tensor.matmul(
                                pv1[:, isl], v_aug[jt][:, h1 * 65:h1 * 65 + 65],
                                pT1[:, isl], start=(jt == 0), stop=(jt == IT - 1))
                    # normalization: recip of column sums (partition 64,
                    # same-lane), gpsimd-broadcast to partitions 0..63,
                    # multiply; odd head moved into place via SBUF->SBUF DMA
                    # (DVE cannot cross partitions).
                    rcp0 = prb.tile([65, N], f32, tag="nrm", name=f"rcp0_{hp}")
                    rcp1 = prb.tile([65, N], f32, tag="nrm", name=f"rcp1_{hp}")
                    nc.vector.reciprocal(out=rcp0[64:65, :], in_=pv0[64:65, :])
                    nc.vector.reciprocal(out=rcp1[64:65, :], in_=pv1[64:65, :])
                    # broadcast via DRAM round-trip (partition_broadcast's ISA
                    # encoding is rejected by this walrus; SBUF APs cannot
                    # have zero partition step, DRAM APs can)
                    ds0 = pdram.tile([1, N], f32, tag="ds", name=f"ds0_{hp}")
                    ds1 = pdram.tile([1, N], f32, tag="ds", name=f"ds1_{hp}")
                    nc.sync.dma_start(out=ds0, in_=rcp0[64:65, :])
                    nc.sync.dma_start(out=ds1, in_=rcp1[64:65, :])
                    rb0 = prb.tile([64, N], f32, tag="nrm", name=f"rb0_{hp}")
                    rb1 = prb.tile([64, N], f32, tag="nrm", name=f"rb1_{hp}")
                    nc.sync.dma_start(
                        out=rb0, in_=bass.AP(tensor=ds0.tensor, offset=ds0.offset,
                                             ap=[[0, 64], [1, N]]))
                    nc.sync.dma_start(
                        out=rb1, in_=bass.AP(tensor=ds1.tensor, offset=ds1.offset,
                                             ap=[[0, 64], [1, N]]))
                    nc.vector.tensor_mul(ovT[hp][0:64, :], pv0[0:64, :], rb0)
                    tmp1 = prb.tile([64, N], bf16, tag="nrm", name=f"tmp1_{hp}")
                    nc.vector.tensor_mul(tmp1, pv1[0:64, :], rb1)
                    nc.sync.dma_start(out=ovT[hp][64:128, :], in_=tmp1)

                # ---- phase 3: y = ovT^T @ w_proj + b_out
                for it in range(IT):
                    isl = slice(it * 128, (it + 1) * 128)
                    pyt = ps.tile([128, E], f32, tag="ps", name=f"py_{it}")
                    for (n0, nw) in ((0, 512), (512, 256)):
                        for e in range(ET):
                            nc.tensor.matmul(
                                pyt[:, n0:n0 + nw],
                                ovT[e][:, isl],
                                wp[e][:, n0:n0 + nw],
                                start=(e == 0), stop=(e == ET - 1))
                    ysb = py.tile([128, E], f32, tag="y", name=f"y{it}")
                    nc.vector.tensor_add(ysb, pyt, bo)
                    nc.sync.dma_start(out=y_d[isl, :], in_=ysb)

    _split_excess_waits(nc)
    return nc


def _get_nc():
    if not _NC_CACHE:
        _NC_CACHE.append(_build_nc())
    return _NC_CACHE[0]


# ---------------------------------------------------------------- entry point
def kernel(x, w_qkv, b_qkv, w_proj, b_proj, _trace=False):
    from concourse.bass_utils import run_bass_kernel_spmd

    import ml_dtypes
    bf16 = ml_dtypes.bfloat16
    x = np.asarray(x)
    w_qk, b_qk, w_v, b_out = _prep_weights(
        np.asarray(w_qkv), np.asarray(b_qkv), np.asarray(w_proj),
        np.asarray(b_proj))
    w_qk16 = w_qk.astype(bf16)
    w_v16 = w_v.astype(bf16)
    w_proj16 = np.ascontiguousarray(np.asarray(w_proj)).astype(bf16)

    in_maps = []
    for b in range(B):
        in_maps.append({
            "xT": np.ascontiguousarray(x[b].T).astype(bf16),
            "w_qk": w_qk16,
            "b_qk": b_qk,
            "w_v": w_v16,
            "w_proj": w_proj16,
            "b_out": b_out,
        })

    nc = _get_nc()
    res = run_bass_kernel_spmd(nc, in_maps, core_ids=list(range(B)),
                               trace=_trace)
    out = np.stack([res.results[b]["y"] for b in range(B)]).astype(np.float32)
    if _trace:
        return out, res
    return out



# revision 2
# speedup vs baseline: 1.4759x; 1.4759x over previous
"""Trainium2 Bass kernel for nn_Attention_78554951844258.

Dense 12-head attention block: qkv = x@Wqkv+b; RoPE(q,k); softmax(q k^T/sqrt(d)) v; proj.

Sharding: data-parallel over batch — each of the 8 NeuronCores computes one
batch element end-to-end (no collectives).

Algebraic restructuring (host-side, exact, O(weights)):
  * The reference applies RoPE with seq_dim=1 on [b,h,n,d], so cos/sin depend
    only on (head, dim) — RoPE is a position-independent per-head 64x64 linear
    map M_h that folds into the q/k columns of w_qkv (and biases).
  * The softmax scale 1/sqrt(d) folds into the q weights.
  * The v bias and proj bias fold into a single output bias
    b_out = b_v @ w_proj + b_proj, because softmax rows sum to 1.
  * Softmax max-subtraction is skipped: folded scores are bounded (|S| < ~3),
    exp is safe in fp32 and the result is mathematically identical.

Device schedule (v2 — ACT-saturating per-head pipeline):
  Phase A: q^T/k^T for head pair 0 (w_qk^T-stationary vs x^T moving),
    v = x @ w_v into ones-augmented per-head v_aug tiles, then pair 1.
    Remaining QK pairs are emitted inside phase B as PE gap-filler.
  Phase B (per head h, per j-tile): S^T [j,i] = k_h^T-stationary x q_h^T
    (K=64); exp via one ACT per [128,1024] tile -> bf16 pT; PV accumulate
    [V_h|1]^T-stationary x pT -> pv [65,1024] (row 64 = softmax colsums).
    PSUM: 2 score slots (4 banks) + 1 pv slot (2) + 1 qk/v/proj slot (2).
    Normalization is deferred: pv casts to bf16 immediately (colsum row
    rides along), freeing its PSUM slot; colsum rows DMA to DRAM scratch.
  Phase C: one batched reciprocal of all 12x1024 colsums rearranged to
    [128,96] (DVE), scatter back to DRAM, 0-partition-step broadcast DMAs,
    6 multiplies -> normalized ov^T, then y = ov^T-stationary x w_proj + b.
Matmul operands bf16, fp32 PSUM accumulation.
"""
import numpy as np

NUM_HEADS = 12
E = 768
D = 64
B = 8
N = 1024
HALF = D // 2


def _ensure_axon_hooks():
    """The NTFF profile hook registry module may be missing in a fresh
    container; (re)create it so trace=True profiling degrades gracefully."""
    try:
        import antenv.axon_hooks  # noqa: F401
        return
    except ImportError:
        pass
    try:
        import antenv
        import os
        p = os.path.join(os.path.dirname(antenv.__file__), "axon_hooks.py")
        with open(p, "w") as f:
            f.write(
                "_hook = None\n\n"
                "def set_axon_ntff_profile_hook(hook):\n"
                "    global _hook\n    _hook = hook\n\n"
                "def get_axon_ntff_profile_hook():\n"
                "    return _hook\n")
    except Exception:
        pass


_ensure_axon_hooks()


# ---------------------------------------------------------------- host math
def _rope_matrix():
    """M[h, x, d]: rope(q)[x] = sum_d M[h, x, d] * q[d] (float64)."""
    inv_freq = 1.0 / (10000.0 ** (np.arange(0, D, 2, dtype=np.float64) / D))
    t = np.arange(NUM_HEADS, dtype=np.float64)
    emb = np.concatenate([t[:, None] * inv_freq[None, :]] * 2, axis=-1)  # [H, D]
    cos, sin = np.cos(emb), np.sin(emb)
    M = np.zeros((NUM_HEADS, D, D))
    for h in range(NUM_HEADS):
        for d in range(D):
            M[h, d, d] = cos[h, d]
            if d < HALF:
                M[h, d, d + HALF] = -sin[h, d]
            else:
                M[h, d, d - HALF] = sin[h, d]
    return M


def _prep_weights(w_qkv, b_qkv, w_proj, b_proj):
    w = w_qkv.astype(np.float64)
    b = b_qkv.astype(np.float64)
    M = _rope_matrix()
    scale = float(D) ** (-0.5)
    w_q = w[:, 0:E].reshape(E, NUM_HEADS, D)
    w_k = w[:, E:2 * E].reshape(E, NUM_HEADS, D)
    b_q = b[0:E].reshape(NUM_HEADS, D)
    b_k = b[E:2 * E].reshape(NUM_HEADS, D)
    w_q2 = np.einsum('ehd,hxd->ehx', w_q, M) * scale
    b_q2 = np.einsum('hd,hxd->hx', b_q, M) * scale
    w_k2 = np.einsum('ehd,hxd->ehx', w_k, M)
    b_k2 = np.einsum('hd,hxd->hx', b_k, M)
    w_qk = np.ascontiguousarray(
        np.concatenate([w_q2.reshape(E, E), w_k2.reshape(E, E)], axis=1),
        dtype=np.float32)                                     # [E, 2E]
    b_qk = np.concatenate([b_q2.reshape(E), b_k2.reshape(E)]).astype(np.float32)
    w_v = np.ascontiguousarray(w[:, 2 * E:3 * E], dtype=np.float32)
    b_out = (b[2 * E:3 * E] @ w_proj.astype(np.float64)
             + b_proj.astype(np.float64)).astype(np.float32)
    return w_qk, b_qk, w_v, b_out


# ---------------------------------------------------------------- waitfix
def _split_excess_waits(nc):
    """walrus in this container rejects >4 sync waits per instruction (and
    fewer on Drain/SP-NoOp paths). Split overflow waits onto preceding
    same-engine 1-wait NOPs — semantically identical (sequencer blocks in
    order)."""
    import concourse.mybir as mybir
    import bass_rust
    counter = [0]

    def make_nop(engine):
        counter[0] += 1
        nop = bass_rust.InstNoOp(name=f"I-waitfix-{counter[0]}", ins=[], outs=[])
        nop.engine = engine
        return nop

    for fn in nc.m.functions:
        for bb in fn.blocks:
            insts = bb.instructions
            out = []
            changed = False
            for inst in insts:
                si = inst.sync_info
                waits = list(si.on_wait) if si is not None else []
                tn = type(inst).__name__
                keep = 0 if tn == "InstDrain" else 1
                if len(waits) > keep:
                    for w in waits[:len(waits) - keep]:
                        nop = make_nop(inst.engine)
                        nop.sync_info = mybir.SyncInfo(on_wait=[w], on_update=[])
                        out.append(nop)
                    inst.sync_info = mybir.SyncInfo(
                        on_wait=waits[len(waits) - keep:],
                        on_update=list(si.on_update))
                    changed = True
                out.append(inst)
            if changed:
                bb.instructions = out


# ---------------------------------------------------------------- device IR
_NC_CACHE = []


def _build_nc():
    import concourse.bass as bass
    import concourse.mybir as mybir
    from concourse.tile import TileContext

    dt = mybir.dt
    f32 = dt.float32
    bf16 = dt.bfloat16
    AF = mybir.ActivationFunctionType

    nc = bass.Bass(target_bir_lowering=False)
    xT_d = nc.dram_tensor("xT", [E, N], bf16, kind="ExternalInput")
    wqk_d = nc.dram_tensor("w_qk", [E, 2 * E], bf16, kind="ExternalInput")
    bqk_d = nc.dram_tensor("b_qk", [2 * E], f32, kind="ExternalInput")
    wv_d = nc.dram_tensor("w_v", [E, E], bf16, kind="ExternalInput")
    wp_d = nc.dram_tensor("w_proj", [E, E], bf16, kind="ExternalInput")
    bo_d = nc.dram_tensor("b_out", [E], f32, kind="ExternalInput")
    y_d = nc.dram_tensor("y", [N, E], f32, kind="ExternalOutput")

    ET = E // 128          # 6 e-tiles
    IT = N // 128          # 8 i/j-tiles
    HP = NUM_HEADS // 2    # 6 head pairs

    with TileContext(nc) as tc:
        with (
            tc.tile_pool(name="persist", bufs=1) as pp,      # weights etc
            tc.tile_pool(name="qkT", bufs=1) as pqk,         # all q^T/k^T
            tc.tile_pool(name="vaug", bufs=1) as pva,
            tc.tile_pool(name="pT", bufs=8) as ppT,          # exp outputs
            tc.tile_pool(name="t65", bufs=3) as ptmp,        # pv casts
            tc.tile_pool(name="ov", bufs=1) as pov,          # ovT / ovT2
            tc.tile_pool(name="rb", bufs=2) as prb,          # recip broadcast
            tc.tile_pool(name="cs", bufs=1) as pcs,
            tc.tile_pool(name="yst", bufs=2) as pys,
            tc.tile_pool(name="dscr", bufs=1, space="DRAM") as pdram,
            tc.tile_pool(name="ps_st", bufs=2, space="PSUM") as stp,
            tc.tile_pool(name="ps_pv", bufs=1, space="PSUM") as pvp,
            tc.tile_pool(name="ps_qk", bufs=1, space="PSUM") as qkp,
        ):
            # ---- loads
            xT = [pp.tile([128, N], bf16, tag=f"xT{e}", name=f"xT{e}")
                  for e in range(ET)]
            wqk = [pp.tile([128, 2 * E], bf16, tag=f"wqk{e}", name=f"wqk{e}")
                   for e in range(ET)]
            wv = [pp.tile([128, E], bf16, tag=f"wv{e}", name=f"wv{e}")
                  for e in range(ET)]
            wp = [pp.tile([128, E], bf16, tag=f"wp{e}", name=f"wp{e}")
                  for e in range(ET)]
            for e in range(ET):
                nc.sync.dma_start(out=xT[e], in_=xT_d[e * 128:(e + 1) * 128, :])
                nc.sync.dma_start(out=wqk[e], in_=wqk_d[e * 128:(e + 1) * 128, :])
                nc.sync.dma_start(out=wv[e], in_=wv_d[e * 128:(e + 1) * 128, :])
                nc.sync.dma_start(out=wp[e], in_=wp_d[e * 128:(e + 1) * 128, :])
            bq = pp.tile([128, 12], f32, tag="bq")
            nc.sync.dma_start(out=bq, in_=bqk_d[:].rearrange("(t p) -> p t", p=128))
            bo = pp.tile([128, E], f32, tag="bo")
            nc.sync.dma_start(
                out=bo,
                in_=bass.AP(tensor=bo_d[:].tensor, offset=bo_d[:].offset,
                            ap=[[0, 128], [1, E]]))

            # warm the ACT exp table early (overlaps phase A); keep it live
            # with a tiny DMA so DCE can't drop it.
            dummy = pcs.tile([128, 12], bf16, tag="dummy")
            nc.scalar.activation(out=dummy, in_=bq, func=AF.Exp)
            dummy_d = pdram.tile([128, 12], bf16, tag="dummy_d")
            nc.sync.dma_start(out=dummy_d, in_=dummy)

            qt = [pqk.tile([128, N], bf16, tag=f"qt{p}", name=f"qt{p}")
                  for p in range(HP)]
            kt = [pqk.tile([128, N], bf16, tag=f"kt{p}", name=f"kt{p}")
                  for p in range(HP)]
            v_aug = [pva.tile([128, NUM_HEADS * (D + 1)], bf16, tag=f"vaug{i}",
                              name=f"vaug{i}") for i in range(IT)]
            ovT = [pov.tile([128, N], bf16, tag=f"ovT{e}", name=f"ovT{e}")
                   for e in range(ET)]
            ovT2 = [pov.tile([128, N], bf16, tag=f"ovT2{e}", name=f"ovT2{e}")
                    for e in range(ET)]
            cs_d = pdram.tile([NUM_HEADS, N], bf16, tag="cs_d")
            rcp_d = pdram.tile([NUM_HEADS * N], f32, tag="rcp_d")

            def emit_qk_pair(p):
                """q^T,k^T for head pair p -> qt[p], kt[p] (2 heads on 128
                partitions)."""
                for ct, dst in ((p, qt[p]), (HP + p, kt[p])):
                    pq = qkp.tile([128, N], f32, tag="qk", name=f"pq{ct}")
                    for ih in range(2):
                        for e in range(ET):
                            nc.tensor.matmul(
                                pq[:, ih * 512:(ih + 1) * 512],
                                wqk[e][:, ct * 128:(ct + 1) * 128],
                                xT[e][:, ih * 512:(ih + 1) * 512],
                                start=(e == 0), stop=(e == ET - 1))
                    nc.vector.tensor_scalar_add(dst, pq, bq[:, ct:ct + 1])

            emit_qk_pair(0)

            # ---- v = x @ w_v, per-head columns with trailing ones column
            for it in range(IT):
                pool, tag = (qkp, "qk") if it % 2 == 0 else (pvp, "pv")
                pv_ps = pool.tile([128, E], f32, tag=tag, name=f"pvv_{it}")
                for (n0, nw) in ((0, 512), (512, 256)):
                    for e in range(ET):
                        nc.tensor.matmul(
                            pv_ps[:, n0:n0 + nw],
                            xT[e][:, it * 128:(it + 1) * 128],
                            wv[e][:, n0:n0 + nw],
                            start=(e == 0), stop=(e == ET - 1))
                nc.vector.tensor_copy(
                    out=v_aug[it].rearrange("p (h c) -> p h c", c=65)[:, :, 0:64],
                    in_=pv_ps.rearrange("p (h d) -> p h d", d=64))
                ones_cols = v_aug[it].rearrange(
                    "p (h c) -> p h c", c=65)[:, :, 64:65]
                bq12 = bq[:, 0:12].rearrange("p (a b) -> p a b", b=1)
                nc.vector.tensor_scalar(
                    ones_cols, bq12, 0.0, 1.0,
                    mybir.AluOpType.mult, mybir.AluOpType.add)

            emit_qk_pair(1)

            # ---- phase B: per head, scores -> exp -> PV
            for h in range(NUM_HEADS):
                pair, half = h // 2, h % 2
                rows = slice(half * 64, half * 64 + 64)
                if h in (2, 4, 6, 8):
                    emit_qk_pair(h // 2 + 1)
                pv = pvp.tile([65, N], f32, tag="pv", name=f"pv_{h}")
                for jt in range(IT):
                    js = slice(jt * 128, (jt + 1) * 128)
                    st = stp.tile([128, N], f32, tag="st", name=f"st_{h}_{jt}")
                    for ih in range(2):
                        isl = slice(ih * 512, (ih + 1) * 512)
                        nc.tensor.matmul(st[:, isl], kt[pair][rows, js],
                                         qt[pair][rows, isl])
                    pT = ppT.tile([128, N], bf16, tag="pT", name=f"pT_{h}_{jt}")
                    nc.scalar.activation(out=pT, in_=st, func=AF.Exp)
                    for ih in range(2):
                        isl = slice(ih * 512, (ih + 1) * 512)
                        nc.tensor.matmul(
                            pv[:, isl], v_aug[jt][:, h * 65:h * 65 + 65],
                            pT[:, isl], start=(jt == 0), stop=(jt == IT - 1))
                # evacuate unnormalized: bf16 cast (colsum row rides along)
                t65 = ptmp.tile([65, N], bf16, tag="t65", name=f"t65_{h}")
                nc.vector.tensor_copy(out=t65, in_=pv)
                nc.sync.dma_start(out=ovT[pair][rows.start:rows.start + 64, :],
                                  in_=t65[0:64, :])
                nc.sync.dma_start(out=cs_d[h:h + 1, :], in_=t65[64:65, :])

            # ---- phase C: batched reciprocal + broadcast + normalize
            cs_sb = pcs.tile([128, 96], bf16, tag="cs_sb")
            nc.sync.dma_start(
                out=cs_sb,
                in_=bass.AP(tensor=cs_d.tensor, offset=cs_d.offset,
                            ap=[[96, 128], [1, 96]]))
            cs_f = pcs.tile([128, 96], f32, tag="cs_f")
            nc.vector.tensor_copy(out=cs_f, in_=cs_sb)
            rcp = pcs.tile([128, 96], f32, tag="rcp")
            nc.vector.reciprocal(out=rcp, in_=cs_f)
            nc.sync.dma_start(
                out=bass.AP(tensor=rcp_d.tensor, offset=rcp_d.offset,
                            ap=[[96, 128], [1, 96]]),
                in_=rcp)
            for p in range(HP):
                rb = prb.tile([128, N], f32, tag="rb", name=f"rb{p}")
                for half in range(2):
                    h = 2 * p + half
                    nc.sync.dma_start(
                        out=rb[half * 64:half * 64 + 64, :],
                        in_=bass.AP(tensor=rcp_d.tensor,
                                    offset=rcp_d.offset + h * N,
                                    ap=[[0, 64], [1, N]]))
                nc.vector.tensor_mul(ovT2[p], ovT[p], rb)

            # ---- proj: y = ovT2^T @ w_proj + b_out
            for it in range(IT):
                isl = slice(it * 128, (it + 1) * 128)
                pool, tag = (stp, "st") if it % 2 == 0 else (qkp, "qk")
                pyt = pool.tile([128, E], f32, tag=tag, name=f"py_{it}")
                for (n0, nw) in ((0, 512), (512, 256)):
                    for e in range(ET):
                        nc.tensor.matmul(
                            pyt[:, n0:n0 + nw],
                            ovT2[e][:, isl],
                            wp[e][:, n0:n0 + nw],
                            start=(e == 0), stop=(e == ET - 1))
                ysb = pys.tile([128, E], f32, tag="y", name=f"y{it}")
                nc.vector.tensor_add(ysb, pyt, bo)
                nc.sync.dma_start(out=y_d[isl, :], in_=ysb)

    _split_excess_waits(nc)
    return nc


def _get_nc():
    if not _NC_CACHE:
        _NC_CACHE.append(_build_nc())
    return _NC_CACHE[0]


# ---------------------------------------------------------------- entry point
def kernel(x, w_qkv, b_qkv, w_proj, b_proj, _trace=False):
    from concourse.bass_utils import run_bass_kernel_spmd

    import ml_dtypes
    bf16 = ml_dtypes.bfloat16
    x = np.asarray(x)
    w_qk, b_qk, w_v, b_out = _prep_weights(
        np.asarray(w_qkv), np.asarray(b_qkv), np.asarray(w_proj),
        np.asarray(b_proj))
    w_qk16 = w_qk.astype(bf16)
    w_v16 = w_v.astype(bf16)
    w_proj16 = np.ascontiguousarray(np.asarray(w_proj)).astype(bf16)

    in_maps = []
    for b in range(B):
        in_maps.append({
            "xT": np.ascontiguousarray(x[b].T).astype(bf16),
            "w_qk": w_qk16,
            "b_qk": b_qk,
            "w_v": w_v16,
            "w_proj": w_proj16,
            "b_out": b_out,
        })

    nc = _get_nc()
    res = run_bass_kernel_spmd(nc, in_maps, core_ids=list(range(B)),
                               trace=_trace)
    out = np.stack([res.results[b]["y"] for b in range(B)]).astype(np.float32)
    if _trace:
        return out, res
    return out


# revision 3
# speedup vs baseline: 1.5469x; 1.0481x over previous
"""Trainium2 Bass kernel for nn_Attention_78554951844258.

Dense 12-head attention block: qkv = x@Wqkv+b; RoPE(q,k); softmax(q k^T/sqrt(d)) v; proj.

Sharding: data-parallel over batch — each of the 8 NeuronCores computes one
batch element end-to-end (no collectives).

Algebraic restructuring (host-side, exact, O(weights)):
  * RoPE here depends only on (head, dim) (seq_dim=1 quirk) — a per-head 64x64
    linear map folded into the q/k columns of w_qkv (and biases); softmax
    scale folded into q; v/proj biases folded into one output bias.
  * Softmax max-subtraction skipped: folded scores are bounded (|S| < ~3).

Device schedule (v3 — ACT-saturating per-head pipeline):
  Phase A: q^T/k^T head pair 0 (w_qk column-split loads arrive first),
    v = x @ w_v into ones-augmented v_aug, pair 1; pairs 2-5 emitted inside
    phase B as PE gap-filler. w_proj loads deferred to late.
  Phase B (per head h, per j-tile pair): S^T = k^T-stationary x q^T (K=64,
    alternating PE row groups via swapped-copy q/k tiles so adjacent j-tiles
    can overlap in the array); exp via one ACT per [128,1024] tile; PV
    accumulate [V_h|1]^T x pT -> pv[65,1024] (row 64 = colsums).
    PSUM: 2 score slots + 1 pv slot + 1 qk/v/proj slot = 8 banks.
    pv casts to bf16 immediately (colsum row rides along; frees PSUM);
    per-PAIR normalization chain (colsums -> DRAM -> [128,16] gather ->
    reciprocal -> scatter -> 0-step broadcast -> multiply) overlaps phase B.
  Phase C: proj accumulates pairs 0..4 first, pair 5 last, overlapping the
    final normalization chain.
Matmul operands bf16, fp32 PSUM accumulation.
"""
import numpy as np

NUM_HEADS = 12
E = 768
D = 64
B = 8
N = 1024
HALF = D // 2


def _ensure_axon_hooks():
    """The NTFF profile hook registry module may be missing in a fresh
    container; (re)create it so trace=True profiling degrades gracefully."""
    try:
        import antenv.axon_hooks  # noqa: F401
        return
    except ImportError:
        pass
    try:
        import antenv
        import os
        p = os.path.join(os.path.dirname(antenv.__file__), "axon_hooks.py")
        with open(p, "w") as f:
            f.write(
                "_hook = None\n\n"
                "def set_axon_ntff_profile_hook(hook):\n"
                "    global _hook\n    _hook = hook\n\n"
                "def get_axon_ntff_profile_hook():\n"
                "    return _hook\n")
    except Exception:
        pass


_ensure_axon_hooks()


# ---------------------------------------------------------------- host math
def _rope_matrix():
    """M[h, x, d]: rope(q)[x] = sum_d M[h, x, d] * q[d] (float64)."""
    inv_freq = 1.0 / (10000.0 ** (np.arange(0, D, 2, dtype=np.float64) / D))
    t = np.arange(NUM_HEADS, dtype=np.float64)
    emb = np.concatenate([t[:, None] * inv_freq[None, :]] * 2, axis=-1)  # [H, D]
    cos, sin = np.cos(emb), np.sin(emb)
    M = np.zeros((NUM_HEADS, D, D))
    for h in range(NUM_HEADS):
        for d in range(D):
            M[h, d, d] = cos[h, d]
            if d < HALF:
                M[h, d, d + HALF] = -sin[h, d]
            else:
                M[h, d, d - HALF] = sin[h, d]
    return M


def _prep_weights(w_qkv, b_qkv, w_proj, b_proj):
    w = w_qkv.astype(np.float64)
    b = b_qkv.astype(np.float64)
    M = _rope_matrix()
    scale = float(D) ** (-0.5)
    w_q = w[:, 0:E].reshape(E, NUM_HEADS, D)
    w_k = w[:, E:2 * E].reshape(E, NUM_HEADS, D)
    b_q = b[0:E].reshape(NUM_HEADS, D)
    b_k = b[E:2 * E].reshape(NUM_HEADS, D)
    w_q2 = np.einsum('ehd,hxd->ehx', w_q, M) * scale
    b_q2 = np.einsum('hd,hxd->hx', b_q, M) * scale
    w_k2 = np.einsum('ehd,hxd->ehx', w_k, M)
    b_k2 = np.einsum('hd,hxd->hx', b_k, M)
    w_qk = np.ascontiguousarray(
        np.concatenate([w_q2.reshape(E, E), w_k2.reshape(E, E)], axis=1),
        dtype=np.float32)                                     # [E, 2E]
    b_qk = np.concatenate([b_q2.reshape(E), b_k2.reshape(E)]).astype(np.float32)
    w_v = np.ascontiguousarray(w[:, 2 * E:3 * E], dtype=np.float32)
    b_out = (b[2 * E:3 * E] @ w_proj.astype(np.float64)
             + b_proj.astype(np.float64)).astype(np.float32)
    return w_qk, b_qk, w_v, b_out


# ---------------------------------------------------------------- waitfix
def _split_excess_waits(nc):
    """walrus in this container rejects >4 sync waits per instruction (and
    fewer on Drain/SP-NoOp paths). Split overflow waits onto preceding
    same-engine 1-wait NOPs — semantically identical (sequencer blocks in
    order)."""
    import concourse.mybir as mybir
    import bass_rust
    counter = [0]

    def make_nop(engine):
        counter[0] += 1
        nop = bass_rust.InstNoOp(name=f"I-waitfix-{counter[0]}", ins=[], outs=[])
        nop.engine = engine
        return nop

    for fn in nc.m.functions:
        for bb in fn.blocks:
            insts = bb.instructions
            out = []
            changed = False
            for inst in insts:
                si = inst.sync_info
                waits = list(si.on_wait) if si is not None else []
                tn = type(inst).__name__
                keep = 0 if tn == "InstDrain" else 1
                if len(waits) > keep:
                    for w in waits[:len(waits) - keep]:
                        nop = make_nop(inst.engine)
                        nop.sync_info = mybir.SyncInfo(on_wait=[w], on_update=[])
                        out.append(nop)
                    inst.sync_info = mybir.SyncInfo(
                        on_wait=waits[len(waits) - keep:],
                        on_update=list(si.on_update))
                    changed = True
                out.append(inst)
            if changed:
                bb.instructions = out


# ---------------------------------------------------------------- device IR
_NC_CACHE = []


def _build_nc():
    import concourse.bass as bass
    import concourse.mybir as mybir
    from concourse.tile import TileContext

    dt = mybir.dt
    f32 = dt.float32
    bf16 = dt.bfloat16
    AF = mybir.ActivationFunctionType

    nc = bass.Bass(target_bir_lowering=False)
    xT_d = nc.dram_tensor("xT", [E, N], bf16, kind="ExternalInput")
    wqk_d = nc.dram_tensor("w_qk", [E, 2 * E], bf16, kind="ExternalInput")
    bqk_d = nc.dram_tensor("b_qk", [2 * E], f32, kind="ExternalInput")
    wv_d = nc.dram_tensor("w_v", [E, E], bf16, kind="ExternalInput")
    wp_d = nc.dram_tensor("w_proj", [E, E], bf16, kind="ExternalInput")
    bo_d = nc.dram_tensor("b_out", [E], f32, kind="ExternalInput")
    y_d = nc.dram_tensor("y", [N, E], f32, kind="ExternalOutput")

    ET = E // 128          # 6 e-tiles
    IT = N // 128          # 8 i/j-tiles
    HP = NUM_HEADS // 2    # 6 head pairs

    with TileContext(nc) as tc:
        with (
            tc.tile_pool(name="persist", bufs=1) as pp,      # weights etc
            tc.tile_pool(name="qkT", bufs=1) as pqk,         # q^T/k^T (+swaps)
            tc.tile_pool(name="vaug", bufs=1) as pva,
            tc.tile_pool(name="pT", bufs=8) as ppT,          # exp outputs
            tc.tile_pool(name="t65", bufs=3) as ptmp,        # pv casts
            tc.tile_pool(name="ov", bufs=1) as pov,          # ovT / ovT2
            tc.tile_pool(name="rb", bufs=2) as prb,          # recip broadcast
            tc.tile_pool(name="cs", bufs=3) as pcs,
            tc.tile_pool(name="yst", bufs=2) as pys,
            tc.tile_pool(name="dscr", bufs=1, space="DRAM") as pdram,
            tc.tile_pool(name="ps_st", bufs=2, space="PSUM") as stp,
            tc.tile_pool(name="ps_pv", bufs=1, space="PSUM") as pvp,
            tc.tile_pool(name="ps_qk", bufs=1, space="PSUM") as qkp,
        ):
            # ---- loads (priority order: xT, bq/bo, wqk pair0, wv, wqk
            # pairs 1-5, wp last)
            xT = [pp.tile([128, N], bf16, tag=f"xT{e}", name=f"xT{e}")
                  for e in range(ET)]
            # w_qk column-split per 128-col block: wqkt[e][ct], ct 0-5 = q
            # pairs, 6-11 = k pairs
            wqkt = [[pp.tile([128, 128], bf16, tag=f"wqk{e}_{ct}",
                             name=f"wqk{e}_{ct}") for ct in range(12)]
                    for e in range(ET)]
            wv = [pp.tile([128, E], bf16, tag=f"wv{e}", name=f"wv{e}")
                  for e in range(ET)]
            wp = [pp.tile([128, E], bf16, tag=f"wp{e}", name=f"wp{e}")
                  for e in range(ET)]
            for e in range(ET):
                nc.sync.dma_start(out=xT[e], in_=xT_d[e * 128:(e + 1) * 128, :])
            bq = pp.tile([128, 12], f32, tag="bq")
            nc.sync.dma_start(out=bq, in_=bqk_d[:].rearrange("(t p) -> p t", p=128))
            bo = pp.tile([128, E], f32, tag="bo")
            nc.sync.dma_start(
                out=bo,
                in_=bass.AP(tensor=bo_d[:].tensor, offset=bo_d[:].offset,
                            ap=[[0, 128], [1, E]]))

            def load_wqk_pair(p):
                for e in range(ET):
                    for ct in (p, HP + p):
                        nc.sync.dma_start(
                            out=wqkt[e][ct],
                            in_=wqk_d[e * 128:(e + 1) * 128,
                                      ct * 128:(ct + 1) * 128])

            load_wqk_pair(0)
            for e in range(ET):
                nc.sync.dma_start(out=wv[e], in_=wv_d[e * 128:(e + 1) * 128, :])
            for p in range(1, HP):
                load_wqk_pair(p)
            for e in range(ET):
                nc.sync.dma_start(out=wp[e], in_=wp_d[e * 128:(e + 1) * 128, :])

            # warm the ACT exp table early; tiny DMA keeps it live.
            dummy = pcs.tile([128, 12], bf16, tag="dummy")
            nc.scalar.activation(out=dummy, in_=bq, func=AF.Exp)
            dummy_d = pdram.tile([128, 12], bf16, tag="dummy_d")
            nc.sync.dma_start(out=dummy_d, in_=dummy)

            # qt/kt pair tiles + swapped copies (heads on opposite 64-row
            # halves) so adjacent j-tile score matmuls hit disjoint PE row
            # groups and can overlap in the array.
            qt = [pqk.tile([128, N], bf16, tag=f"qt{p}", name=f"qt{p}")
                  for p in range(HP)]
            kt = [pqk.tile([128, N], bf16, tag=f"kt{p}", name=f"kt{p}")
                  for p in range(HP)]
            qts = [pqk.tile([128, N], bf16, tag=f"qts{p}", name=f"qts{p}")
                   for p in range(HP)]
            kts = [pqk.tile([128, N], bf16, tag=f"kts{p}", name=f"kts{p}")
                   for p in range(HP)]
            v_aug = [pva.tile([128, NUM_HEADS * (D + 1)], bf16, tag=f"vaug{i}",
                              name=f"vaug{i}") for i in range(IT)]
            ovT = [pov.tile([128, N], bf16, tag=f"ovT{e}", name=f"ovT{e}")
                   for e in range(ET)]
            ovT2 = [pov.tile([128, N], bf16, tag=f"ovT2{e}", name=f"ovT2{e}")
                    for e in range(ET)]
            cs_d = pdram.tile([NUM_HEADS, N], bf16, tag="cs_d")
            rcp_d = pdram.tile([NUM_HEADS * N], f32, tag="rcp_d")

            def emit_qk_pair(p):
                """q^T,k^T for head pair p -> qt[p], kt[p] + swapped copies."""
                for ct, dst, dsw in ((p, qt[p], qts[p]),
                                     (HP + p, kt[p], kts[p])):
                    pq = qkp.tile([128, N], f32, tag="qk", name=f"pq{ct}")
                    for e in range(ET):
                        for ih in range(2):
                            nc.tensor.matmul(
                                pq[:, ih * 512:(ih + 1) * 512],
                                wqkt[e][ct],
                                xT[e][:, ih * 512:(ih + 1) * 512],
                                start=(e == 0), stop=(e == ET - 1))
                    nc.vector.tensor_scalar_add(dst, pq, bq[:, ct:ct + 1])
                    nc.sync.dma_start(out=dsw[64:128, :], in_=dst[0:64, :])
                    nc.sync.dma_start(out=dsw[0:64, :], in_=dst[64:128, :])

            emit_qk_pair(0)

            # ---- v = x @ w_v, per-head columns with trailing ones column
            for it in range(IT):
                pool, tag = (qkp, "qk") if it % 2 == 0 else (pvp, "pv")
                pv_ps = pool.tile([128, E], f32, tag=tag, name=f"pvv_{it}")
                for (n0, nw) in ((0, 512), (512, 256)):
                    for e in range(ET):
                        nc.tensor.matmul(
                            pv_ps[:, n0:n0 + nw],
                            xT[e][:, it * 128:(it + 1) * 128],
                            wv[e][:, n0:n0 + nw],
                            start=(e == 0), stop=(e == ET - 1))
                nc.vector.tensor_copy(
                    out=v_aug[it].rearrange("p (h c) -> p h c", c=65)[:, :, 0:64],
                    in_=pv_ps.rearrange("p (h d) -> p h d", d=64))
                ones_cols = v_aug[it].rearrange(
                    "p (h c) -> p h c", c=65)[:, :, 64:65]
                bq12 = bq[:, 0:12].rearrange("p (a b) -> p a b", b=1)
                nc.vector.tensor_scalar(
                    ones_cols, bq12, 0.0, 1.0,
                    mybir.AluOpType.mult, mybir.AluOpType.add)

            emit_qk_pair(1)

            def norm_pair(p):
                """Reciprocal + broadcast + normalize for head pair p.
                Colsum rows for heads 2p, 2p+1 are already in cs_d."""
                csp = pcs.tile([128, 16], bf16, tag="csp", name=f"csp{p}")
                nc.sync.dma_start(
                    out=csp,
                    in_=bass.AP(tensor=cs_d.tensor,
                                offset=cs_d.offset + p * 2048,
                                ap=[[16, 128], [1, 16]]))
                csf = pcs.tile([128, 16], f32, tag="csf", name=f"csf{p}")
                nc.vector.tensor_copy(out=csf, in_=csp)
                rcp = pcs.tile([128, 16], f32, tag="rcp", name=f"rcp{p}")
                nc.vector.reciprocal(out=rcp, in_=csf)
                nc.sync.dma_start(
                    out=bass.AP(tensor=rcp_d.tensor,
                                offset=rcp_d.offset + p * 2048,
                                ap=[[16, 128], [1, 16]]),
                    in_=rcp)
                rb = prb.tile([128, N], f32, tag="rb", name=f"rb{p}")
                for half in range(2):
                    nc.sync.dma_start(
                        out=rb[half * 64:half * 64 + 64, :],
                        in_=bass.AP(tensor=rcp_d.tensor,
                                    offset=rcp_d.offset + (2 * p + half) * N,
                                    ap=[[0, 64], [1, N]]))
                nc.vector.tensor_mul(ovT2[p], ovT[p], rb)

            # ---- phase B: per head; j-tiles in pairs (even j-tile on rows
            # 0:64, odd on rows 64:128 via swapped tiles)
            for h in range(NUM_HEADS):
                pair, half = h // 2, h % 2
                if h in (2, 4, 6, 8):
                    emit_qk_pair(h // 2 + 1)
                pv = pvp.tile([65, N], f32, tag="pv", name=f"pv_{h}")
                for jp in range(IT // 2):
                    sts = []
                    for sub in range(2):
                        jt = 2 * jp + sub
                        js = slice(jt * 128, (jt + 1) * 128)
                        # rows parity for this jt: even jt -> rows 0:64,
                        # odd jt -> rows 64:128
                        par = jt % 2
                        if par == half:
                            ksrc, qsrc = kt[pair], qt[pair]
                        else:
                            ksrc, qsrc = kts[pair], qts[pair]
                        rows = slice(par * 64, par * 64 + 64)
                        sts.append((jt, js, ksrc, qsrc, rows))
                    st_t = [stp.tile([128, N], f32, tag="st",
                                     name=f"st_{h}_{2*jp+s}") for s in range(2)]
                    for ih in range(2):
                        isl = slice(ih * 512, (ih + 1) * 512)
                        for s, (jt, js, ksrc, qsrc, rows) in enumerate(sts):
                            nc.tensor.matmul(st_t[s][:, isl], ksrc[rows, js],
                                             qsrc[rows, isl])
                    pTs = []
                    for s, (jt, js, ksrc, qsrc, rows) in enumerate(sts):
                        pT = ppT.tile([128, N], bf16, tag="pT",
                                      name=f"pT_{h}_{jt}")
                        nc.scalar.activation(out=pT, in_=st_t[s], func=AF.Exp)
                        pTs.append(pT)
                    for s, (jt, js, ksrc, qsrc, rows) in enumerate(sts):
                        for ih in range(2):
                            isl = slice(ih * 512, (ih + 1) * 512)
                            nc.tensor.matmul(
                                pv[:, isl], v_aug[jt][:, h * 65:h * 65 + 65],
                                pTs[s][:, isl],
                                start=(jt == 0), stop=(jt == IT - 1))
                # evacuate unnormalized: bf16 cast (colsum row rides along)
                t65 = ptmp.tile([65, N], bf16, tag="t65", name=f"t65_{h}")
                nc.vector.tensor_copy(out=t65, in_=pv)
                nc.sync.dma_start(out=ovT[pair][half * 64:half * 64 + 64, :],
                                  in_=t65[0:64, :])
                nc.sync.dma_start(out=cs_d[h:h + 1, :], in_=t65[64:65, :])
                if half == 1:
                    norm_pair(pair)

            # ---- proj: y = ovT2^T @ w_proj + b_out (pair 5 accumulated last)
            e_order = [0, 1, 2, 3, 4, 5]
            for it in range(IT):
                isl = slice(it * 128, (it + 1) * 128)
                pool, tag = (stp, "st") if it % 2 == 0 else (qkp, "qk")
                pyt = pool.tile([128, E], f32, tag=tag, name=f"py_{it}")
                for (n0, nw) in ((0, 512), (512, 256)):
                    for idx, e in enumerate(e_order):
                        nc.tensor.matmul(
                            pyt[:, n0:n0 + nw],
                            ovT2[e][:, isl],
                            wp[e][:, n0:n0 + nw],
                            start=(idx == 0), stop=(idx == ET - 1))
                ysb = pys.tile([128, E], f32, tag="y", name=f"y{it}")
                nc.vector.tensor_add(ysb, pyt, bo)
                nc.sync.dma_start(out=y_d[isl, :], in_=ysb)

    _split_excess_waits(nc)
    return nc


def _get_nc():
    if not _NC_CACHE:
        _NC_CACHE.append(_build_nc())
    return _NC_CACHE[0]


# ---------------------------------------------------------------- entry point
def kernel(x, w_qkv, b_qkv, w_proj, b_proj, _trace=False):
    from concourse.bass_utils import run_bass_kernel_spmd

    import ml_dtypes
    bf16 = ml_dtypes.bfloat16
    x = np.asarray(x)
    w_qk, b_qk, w_v, b_out = _prep_weights(
        np.asarray(w_qkv), np.asarray(b_qkv), np.asarray(w_proj),
        np.asarray(b_proj))
    w_qk16 = w_qk.astype(bf16)
    w_v16 = w_v.astype(bf16)
    w_proj16 = np.ascontiguousarray(np.asarray(w_proj)).astype(bf16)

    in_maps = []
    for b in range(B):
        in_maps.append({
            "xT": np.ascontiguousarray(x[b].T).astype(bf16),
            "w_qk": w_qk16,
            "b_qk": b_qk,
            "w_v": w_v16,
            "w_proj": w_proj16,
            "b_out": b_out,
        })

    nc = _get_nc()
    res = run_bass_kernel_spmd(nc, in_maps, core_ids=list(range(B)),
                               trace=_trace)
    out = np.stack([res.results[b]["y"] for b in range(B)]).astype(np.float32)
    if _trace:
        return out, res
    return out
